# revision 48
# baseline (speedup 1.0000x reference)
"""Tensor-parallel causal multi-head attention (RoPE) for 8 Trainium2 cores.

Problem: nn_Attention (B=2, S=2048, E=2048, H=16, interleaved-pair RoPE,
causal softmax with 1/sqrt(E) scaling, output projection).

Sharding: tensor-parallel over heads — each of the 8 cores owns 2 heads
(the matching 256 columns of Wq/Wk/Wv and rows of Wo), x is replicated,
and the post-Wo all-reduce is done on the host (sum of 8 partials).

Per-core device pipeline (fp32 psum accumulation everywhere):
  1. Q/K projections run in fp8e4m3 with DoubleRow perf mode (2 k-tiles
     of 128 per pass, 0.5 cyc/row — 2x bf16): x and Wq/Wk are staged in
     fp8 (weights pre-scaled by WSCALE=64 into e4m3's normal range; the
     1/WSCALE^2 rides the softmax exp scale).  The fp8 quantization only
     perturbs attention *scores* (~1% of a prob), not the value path.
     V projection stays bf16 from a bf16 copy of x.  Q^T/K^T are kept in
     transposed layout [D, t]; V in natural layout [t, D].
  2. RoPE on Q^T/K^T psums directly: a_t = psum*m1 and c_t = psum*m2'
     (DVE, m2' pre-swapped on host), half-swap of c_t via two Scalar-queue
     DMAs (a compute-dependent DMA on the Sync queue would head-of-line
     block the x prefetches), final add on GpSimd.
  3. Attention per (batch, head, q-tile of 512) with key chunks processed
     in pairs (one [128,1024] exp per pair on ACT; no max-subtraction:
     |scores/sqrt(E)| <~ 1.5).  Diagonal chunks are causally trimmed and
     masked (DVE).  PV accumulates out^T += V_c^T probs^T on PE.  The
     softmax denominator: diagonal chunks use a bf16 ones-matmul; full
     pairs are cast to fp8 (DVE) and reduced by fp8 DoubleRow ones-matmuls
     batched at q-tile end (0.5 cyc/row over both chunks).
  4. Normalization: deferred psum eviction, reciprocal_approx_fast, scale
     in place.  Output projection from out^T (stationary) -> fp16 partial
     [t, E]; host sums the 8 partials in fp64.

Schedule: projection tiles and attention q-tiles are interleaved (attention
(b, qt) only needs K/V coverage up to tile b*4+qt), with output-projection
chunks streaming one q-tile behind attention; x tiles prefetch 2 ahead on
the pure-load Sync queue; psum evictions alternate Scalar/Vector so neither
queue lags the PE; ~5 us of warm-up matmuls ramp the DVFS clock during the
startup DMA window.
"""

import math
import os
from contextlib import ExitStack

import ml_dtypes
import numpy as np

import concourse.bass as bass
import concourse.mybir as mybir
import concourse.tile as tile
from concourse import bacc, bass_isa, bass_utils

# denominator strategy: "pe" = ones-matmul on TensorE into psum;
# "gpsimd" = accumulate exp chunks + partition_all_reduce on GpSimd
DENOM_MODE = os.environ.get("KERNEL_DENOM", "pe")
# partial-output dtype: bf16 halves the output DMA; host sums in fp64
# partial-output dtype fp16: same 2 bytes/elem as bf16 but 10 mantissa
# bits, so the host-side 8-way partial sum loses less precision
OUT_F16 = os.environ.get("KERNEL_OUT", "f16") == "f16"

# ---------------------------------------------------------------- constants
B, S, E = 2, 2048, 2048
H = 16
N_CORES = 8
HPC = H // N_CORES          # heads per core = 2
D = E // H                  # head dim = 128
T = B * S                   # tokens = 4096
HD = HPC * D                # per-core head dims = 256
ATTN_SCALE = 1.0 / math.sqrt(E)
ROPE_BASE = 10000.0

P = 128
EC = E // P                 # 16 contraction chunks
T_TILE = 512
NT = T // T_TILE            # 8 projection token tiles
QTS = 512                   # attention q-tile size
NQT = S // QTS              # 4 q-tiles per (b, h)
NKC = S // P                # 16 key chunks per batch

BF16 = mybir.dt.bfloat16
F32 = mybir.dt.float32
F8 = mybir.dt.float8e4
F16 = mybir.dt.float16
NPBF16 = ml_dtypes.bfloat16
NPF8 = ml_dtypes.float8_e4m3

# Q/K projections run in fp8e4m3 with DoubleRow (2 k-tiles per pass, 0.5
# cyc/row).  Wq/Wk entries (~N(0, 1/E)) are pre-scaled by WSCALE so they
# land in e4m3's normal range; the 1/WSCALE^2 is folded into the exp scale.
WSCALE = 64.0


# ---------------------------------------------------------------- device IR
def _emit(tc, ctx):
    nc = tc.nc
    xTt = nc.dram_tensor("xTt", [NT, P, EC, T_TILE], BF16, kind="ExternalInput").ap()
    x8t = nc.dram_tensor("x8t", [NT, P, EC, T_TILE], F8, kind="ExternalInput").ap()
    wqT = nc.dram_tensor("wqT", [P, EC, HD], F8, kind="ExternalInput").ap()
    wkT = nc.dram_tensor("wkT", [P, EC, HD], F8, kind="ExternalInput").ap()
    wvT = nc.dram_tensor("wvT", [P, EC, HD], BF16, kind="ExternalInput").ap()
    woT = nc.dram_tensor("woT", [P, HPC, E], BF16, kind="ExternalInput").ap()
    rm1 = nc.dram_tensor("rm1", [P, S], BF16, kind="ExternalInput").ap()
    rm2 = nc.dram_tensor("rm2", [P, S], BF16, kind="ExternalInput").ap()
    msk = nc.dram_tensor("msk", [P, QTS], BF16, kind="ExternalInput").ap()
    out = nc.dram_tensor("out", [T, E], F16 if OUT_F16 else F32,
                         kind="ExternalOutput").ap()

    wpool = ctx.enter_context(tc.tile_pool(name="wpool", bufs=1))
    xpool = ctx.enter_context(tc.tile_pool(name="xpool", bufs=2))
    qkv = ctx.enter_context(tc.tile_pool(name="qkv", bufs=1))
    work = ctx.enter_context(tc.tile_pool(name="work", bufs=3))
    psA = ctx.enter_context(tc.tile_pool(name="psA", bufs=2, space="PSUM"))
    psO = ctx.enter_context(tc.tile_pool(name="psO", bufs=2, space="PSUM"))
    psD = ctx.enter_context(tc.tile_pool(name="psD", bufs=2, space="PSUM"))

    # --- persistent SBUF state
    wq_s = wpool.tile([P, EC, HD], F8)
    wk_s = wpool.tile([P, EC, HD], F8)
    wv_s = wpool.tile([P, EC, HD], BF16)
    wo_s = wpool.tile([P, HPC, E], BF16)
    m1_s = wpool.tile([P, S], BF16)
    m2_s = wpool.tile([P, S], BF16)
    mk_s = wpool.tile([P, QTS], BF16)
    ones_s = wpool.tile([P, P], BF16)
    ones8_s = wpool.tile([P, 2, P], F8)   # DoubleRow ones for fp8 denoms
    # startup-latency ordering: few big descriptors (each ~0.6 us of issue
    # time on the Sync queue), fp8 weights/x first so the first Q chain can
    # run as soon as possible
    xt80 = xpool.tile([P, EC, T_TILE], F8, tag="xt8", bufs=3)
    xt0 = xpool.tile([P, EC, T_TILE], BF16, tag="xt")
    nc.sync.dma_start(wq_s[:], wqT[:])
    nc.sync.dma_start(xt80[:], x8t[0])
    nc.sync.dma_start(wk_s[:], wkT[:])
    nc.any.memset(ones_s[:], 1.0)
    nc.any.memset(ones8_s[:], 1.0)
    # HAM warm-up: ~3.5 µs of dummy matmuls during the startup DMA window
    # so the first real matmuls run at the full 2.4 GHz clock
    warm = psA.tile([P, 512], F32, tag="big", bufs=3,
                    padded_shape=[P, 2 * QTS])
    for i in range(56):
        nc.tensor.matmul(warm[:, 0:P], lhsT=ones_s[:], rhs=ones_s[:],
                         start=(i == 0), stop=(i == 55))
    # rope maps: first token-tile's columns land early so the first rope
    # vector ops don't wait on the full 1 MB map load
    nc.sync.dma_start(m1_s[:, 0:T_TILE], rm1[:, 0:T_TILE])
    nc.sync.dma_start(m2_s[:, 0:T_TILE], rm2[:, 0:T_TILE])
    nc.sync.dma_start(xt0[:], xTt[0])
    nc.sync.dma_start(wv_s[:], wvT[:])
    # bulk loads that are needed late (wo only in phase C, msk in phase B,
    # map remainders from tile 1 on) are emitted inside the tile loop so
    # the early x-tile prefetches aren't queued behind them

    qT_s = qkv.tile([P, HPC, T], BF16)   # roped Q^T  [d, h, t]
    kT_s = qkv.tile([P, HPC, T], BF16)   # roped K^T
    v_s = qkv.tile([P, T // P, HD], BF16)  # V natural [t%128, t//128, hd]
    oT_s = qkv.tile([P, HPC, T], BF16)   # normalized out^T [d, h, t]

    # ---------------- phase A: projections + RoPE
    # x tiles are prefetched ahead (fp8 tiles two ahead — they gate the Q/K
    # chains; bf16 one ahead), emitted before tile tt's rope/V work floods
    # the Sync queue
    x8tiles = {0: xt80}
    xtiles = {0: xt0}

    def _pf8(tt):
        if tt < NT and tt not in x8tiles:
            nxt8 = xpool.tile([P, EC, T_TILE], F8, tag="xt8", bufs=3)
            nc.sync.dma_start(nxt8[:], x8t[tt])
            x8tiles[tt] = nxt8

    def _pf16(tt):
        if tt < NT and tt not in xtiles:
            nxt = xpool.tile([P, EC, T_TILE], BF16, tag="xt")
            nc.sync.dma_start(nxt[:], xTt[tt])
            xtiles[tt] = nxt

    def _tileA(tt):
        ts0 = tt * T_TILE
        xt8 = x8tiles.pop(tt)
        xt = xtiles.pop(tt)
        _pf8(tt + 1)
        _pf16(tt + 1)
        _pf8(tt + 2)
        if tt == 0:
            nc.sync.dma_start(m1_s[:, T_TILE:], rm1[:, T_TILE:])
            nc.sync.dma_start(m2_s[:, T_TILE:], rm2[:, T_TILE:])
            nc.sync.dma_start(mk_s[:], msk[:])
        elif tt == 2:
            nc.sync.dma_start(wo_s[:], woT[:])

        for w_s, dst in ((wq_s, qT_s), (wk_s, kT_s)):
            psb = psA.tile([P, 2 * T_TILE], F32, tag="big", bufs=3)
            for hs in range(HPC):
                ps = psb[:, hs * T_TILE:(hs + 1) * T_TILE]
                for e2 in range(EC // 2):
                    nc.tensor.matmul(
                        ps,
                        lhsT=w_s[:, 2 * e2:2 * e2 + 2, hs * P:(hs + 1) * P],
                        rhs=xt8[:, 2 * e2:2 * e2 + 2, :],
                        start=(e2 == 0),
                        stop=(e2 == EC // 2 - 1),
                        perf_mode=mybir.MatmulPerfMode.DoubleRow,
                    )
                # RoPE: e = [x1; x2];  out = e*[cos;cos] + swap64(e)*[-sin;sin]
                #     = e*m1 + swap64(e*m2')   with m2' = swap64(m2)
                # Both multiplies read the psum directly (no eviction copy);
                # the half-swap DMAs ride the Scalar queue right behind their
                # producer (a compute-dependent DMA on the Sync queue would
                # head-of-line block the x prefetches), and the final add
                # runs on the otherwise-idle GpSimd — qT/kT have a full
                # tile of slack before attention reads them.
                ms0 = ts0 % S  # the rope maps are identical across batches
                c_t = work.tile([P, T_TILE], BF16, tag="rope_c")
                nc.vector.tensor_mul(c_t[:], ps, m2_s[:, ms0:ms0 + T_TILE])
                a_t = work.tile([P, T_TILE], BF16, tag="rope_a")
                nc.vector.tensor_mul(a_t[:], ps, m1_s[:, ms0:ms0 + T_TILE])
                b_t = work.tile([P, T_TILE], BF16, tag="rope_b")
                nc.scalar.dma_start(b_t[0:64, :], c_t[64:128, :])
                nc.scalar.dma_start(b_t[64:128, :], c_t[0:64, :])
                nc.gpsimd.tensor_add(
                    out=dst[:, hs, ts0:ts0 + T_TILE], in0=a_t[:], in1=b_t[:])

        for sp in range(T_TILE // P // 2):
            psb = psA.tile([P, 2 * HD], F32, tag="big", bufs=3,
                           padded_shape=[P, 2 * QTS])
            for k in range(2):
                sub = 2 * sp + k
                for ec in range(EC):
                    nc.tensor.matmul(
                        psb[:, k * HD:(k + 1) * HD],
                        lhsT=xt[:, ec, sub * P:(sub + 1) * P],
                        rhs=wv_s[:, ec, :],
                        start=(ec == 0),
                        stop=(ec == EC - 1),
                    )
            nc.scalar.copy(
                v_s[:, tt * (T_TILE // P) + 2 * sp:
                    tt * (T_TILE // P) + 2 * sp + 2, :], psb[:])

    # ---------------- phase C helper: output projection for one 128-token
    # chunk; interleaved into phase B with a one-q-tile lag so the out DMA
    # streams during attention instead of piling up at the end
    def _phaseC_chunk(b, tch, last):
        t0 = b * S + tch * P
        stage = work.tile([P, E], F16 if OUT_F16 else F32, tag="wo_out")
        for ep in range(E // 1024):
            wps = psA.tile([P, 1024], F32, tag="big", bufs=3)
            for k in range(2):
                es = 2 * ep + k
                for hc in range(HPC):
                    nc.tensor.matmul(
                        wps[:, k * 512:(k + 1) * 512],
                        lhsT=oT_s[:, hc, t0:t0 + P],
                        rhs=wo_s[:, hc, es * 512:(es + 1) * 512],
                        start=(hc == 0),
                        stop=(hc == HPC - 1),
                    )
            # alternate eviction engines so neither queue lags the PE
            if ep == 0:
                nc.scalar.copy(stage[:, ep * 1024:(ep + 1) * 1024], wps[:])
            else:
                nc.vector.tensor_copy(
                    out=stage[:, ep * 1024:(ep + 1) * 1024], in_=wps[:])
            if last:
                # drain the final tile per-slice to shorten the tail
                nc.sync.dma_start(
                    out[t0:t0 + P, ep * 1024:(ep + 1) * 1024],
                    stage[:, ep * 1024:(ep + 1) * 1024])
        if not last:
            nc.sync.dma_start(out[t0:t0 + P, :], stage[:])

    pendingC = []  # (b, tch) chunks whose oT is complete but Wo not emitted

    def _flushC(upto):
        while len(pendingC) > upto:
            cb, ctch = pendingC.pop(0)
            _phaseC_chunk(cb, ctch, last=(cb == B - 1 and ctch == S // P - 1))

    # ---------------- phase B: attention for one (batch, head, q-tile)
    def _attn(b, hs, qt):
        qTb = qT_s[:, hs, b * S:(b + 1) * S]
        kTb = kT_s[:, hs, b * S:(b + 1) * S]
        if True:
            if True:
                q0 = qt * QTS
                nck = (q0 + QTS) // P  # causal: key chunks 0..nck-1
                ops = psO.tile([P, QTS], F32, tag="outT", bufs=1)
                if DENOM_MODE == "pe":
                    dps = psD.tile([P, QTS], F32, tag="den", bufs=1)
                else:
                    acc = work.tile([P, QTS], F32, tag="acc", bufs=2)
                # full (untrimmed) pairs contribute to the denominator via a
                # single fp8 DoubleRow ones-matmul per pair (0.5 cyc/row over
                # both chunks), batched after the pair loop; only the 4
                # diagonal chunks keep the bf16 per-chunk ones-matmul.
                # fp8 quantization of exp values is harmless here: the
                # denominator errors are incoherent over >=512 keys.
                dr8s = []
                first_inline = max(0, nck - 4)
                for pp in range(nck // 2):
                    cc = (2 * pp, 2 * pp + 1)
                    # causal trim: diagonal chunk j (=c-(nck-4)) only has
                    # valid queries q >= q0 + 128*j  ->  width 512-128*j
                    jj = [max(0, c - (nck - 4)) for c in cc]
                    off = [128 * j for j in jj]
                    sps = psA.tile([P, 2 * QTS], F32, tag="big", bufs=3)
                    for half, c in enumerate(cc):
                        nc.tensor.matmul(
                            sps[:, half * QTS + off[half]:(half + 1) * QTS],
                            lhsT=kTb[:, c * P:(c + 1) * P],
                            rhs=qTb[:, q0 + off[half]:q0 + QTS],
                            start=True,
                            stop=True,
                        )
                    ex = work.tile([P, 2 * QTS], BF16, tag="exps", bufs=6)
                    exp_scale = ATTN_SCALE / (WSCALE * WSCALE)
                    if off[0] == 0 and off[1] == 0:
                        nc.scalar.activation(
                            ex[:], sps[:], mybir.ActivationFunctionType.Exp,
                            scale=exp_scale,
                        )
                    else:
                        for half in range(2):
                            sl = slice(half * QTS + off[half], (half + 1) * QTS)
                            nc.scalar.activation(
                                ex[:, sl], sps[:, sl],
                                mybir.ActivationFunctionType.Exp,
                                scale=exp_scale,
                            )
                    full_pair = DENOM_MODE == "pe" and cc[1] < nck - 4
                    if full_pair:
                        ex8 = work.tile([P, 2, QTS], F8, tag="exps8", bufs=6)
                        nc.vector.tensor_copy(out=ex8[:, 0, :], in_=ex[:, 0:QTS])
                        nc.vector.tensor_copy(out=ex8[:, 1, :], in_=ex[:, QTS:])
                        dr8s.append(ex8)
                    for half, c in enumerate(cc):
                        w = QTS - off[half]
                        exh = ex[:, half * QTS + off[half]:(half + 1) * QTS]
                        if c >= nck - 4:
                            # intra-block triangle: reuse the j=0 mask, width w
                            nc.vector.tensor_mul(exh, exh, mk_s[:, :w])
                        nc.tensor.matmul(
                            ops[:, off[half]:QTS],
                            lhsT=v_s[:, b * NKC + c, hs * P:(hs + 1) * P],
                            rhs=exh,
                            start=(c == 0),
                            stop=(c == nck - 1),
                        )
                        if DENOM_MODE == "pe":
                            if not full_pair:
                                nc.tensor.matmul(
                                    dps[:, off[half]:QTS],
                                    lhsT=ones_s[:],
                                    rhs=exh,
                                    start=(c == first_inline),
                                    stop=(c == nck - 1 and nck == 4),
                                )
                        elif c == 0:
                            nc.gpsimd.tensor_copy(out=acc[:], in_=exh)
                        else:
                            accs = acc[:, off[half]:]
                            nc.gpsimd.tensor_add(out=accs, in0=accs, in1=exh)
                # batched fp8 DoubleRow denominator matmuls for the full
                # pairs (the inline diag chunks ran first and set start)
                for di, ex8 in enumerate(dr8s):
                    nc.tensor.matmul(
                        dps[:],
                        lhsT=ones8_s[:],
                        rhs=ex8[:],
                        start=False,
                        stop=(di == len(dr8s) - 1),
                        perf_mode=mybir.MatmulPerfMode.DoubleRow,
                        skip_group_check=True,
                    )
                # normalize: oT = ops * (1/denom), denom replicated to all
                # 128 partitions (by the ones-matmul / partition_all_reduce)
                oslice = oT_s[:, hs, b * S + q0: b * S + q0 + QTS]
                rb = work.tile([P, QTS], F32, tag="recipb", bufs=2)
                if DENOM_MODE == "pe":
                    # deferred eviction: the copy frees the psO bank without
                    # waiting on the reciprocal, then scale in place
                    nc.vector.tensor_copy(out=oslice, in_=ops[:])
                    nc.vector.reciprocal_approx_fast(out=rb[:], in_=dps[:])
                    nc.vector.tensor_mul(oslice, oslice, rb[:])
                else:
                    # deferred normalization: release the psum bank with an
                    # unnormalized eviction; scale in place once the (slow,
                    # off-critical-path) GpSimd denominator lands
                    nc.vector.tensor_copy(out=oslice, in_=ops[:])
                    red = work.tile([P, QTS], F32, tag="red")
                    nc.gpsimd.partition_all_reduce(
                        red[:], acc[:], P, bass_isa.ReduceOp.add)
                    nc.vector.reciprocal_approx_fast(out=rb[:], in_=red[:])
                    nc.vector.tensor_mul(oslice, oslice, rb[:])

    # ---------------- schedule: A-tiles interleaved with attention q-tiles
    # attention (b, qt) needs K/V coverage of tiles b*NQT..b*NQT+qt only, so
    # it can start as soon as those projection tiles are done; the early
    # A-tiles' DMA debt is hidden behind attention compute that needs no
    # new input, and phase-C chunks (one q-tile of lag) stream the output
    _tileA(0)
    _tileA(1)
    _tileA(2)
    for b in range(B):
        for qt in range(NQT):
            for hs in range(HPC):
                _attn(b, hs, qt)
            # both heads' oT for tokens [q0, q0+QTS) are now final; queue
            # their output-projection chunks, flush with one q-tile of lag
            # so the normalize has time to land
            pendingC.extend(
                (b, tc) for tc in range(qt * (QTS // P),
                                        (qt + 1) * (QTS // P)))
            _flushC(QTS // P)
            t_next = b * NQT + qt + 3
            if t_next < NT:
                _tileA(t_next)
    _flushC(0)


def build_nc():
    nc = bacc.Bacc("TRN2", target_bir_lowering=False, debug=False, num_devices=1)
    with tile.TileContext(nc) as tc, ExitStack() as ctx:
        _emit(tc, ctx)
    nc.compile()
    return nc


# ---------------------------------------------------------------- host prep
def _rope_maps():
    half = D // 2
    inv = 1.0 / (ROPE_BASE ** (np.arange(half, dtype=np.float64) / half))
    ang = np.arange(S, dtype=np.float64)[None, :] * inv[:, None]  # [64, S]
    cos = np.cos(ang)
    sin = np.sin(ang)
    m1 = np.concatenate([cos, cos], axis=0)   # [128, S] multiplies e=[x1;x2]
    # m2' = swap64(m2): multiplies e BEFORE the half-swap, so that
    # swap64(e*m2') == swap64(e)*m2 with m2 = [-sin; sin]
    m2 = np.concatenate([sin, -sin], axis=0)
    return (np.ascontiguousarray(m1.astype(NPBF16)),
            np.ascontiguousarray(m2.astype(NPBF16)))


def _masks():
    kk = np.arange(P)[:, None]
    qq = np.arange(QTS)[None, :]
    return np.ascontiguousarray((kk <= qq).astype(NPBF16))  # [128, 512]


def _prep_in_maps(x, Wq, Wk, Wv, Wo):
    x = np.asarray(x, np.float32)
    Wq = np.asarray(Wq, np.float32)
    Wk = np.asarray(Wk, np.float32)
    Wv = np.asarray(Wv, np.float32)
    Wo = np.asarray(Wo, np.float32)

    # x^T tiled: [NT, 128, EC, T_TILE];  xT[e, t] = x[t, e]
    xTf = x.reshape(T, E).T                                    # [E, T] f32
    xT = xTf.astype(NPBF16)
    xtt = xT.reshape(EC, P, NT, T_TILE).transpose(2, 1, 0, 3)  # [NT,P,EC,TT]
    xtt = np.ascontiguousarray(xtt)
    x8 = xTf.astype(NPF8)
    x8tt = np.ascontiguousarray(
        x8.reshape(EC, P, NT, T_TILE).transpose(2, 1, 0, 3))

    m1, m2 = _rope_maps()
    msk = _masks()

    # de-interleave perm for RoPE pair-contiguity
    perm = np.concatenate([np.arange(0, D, 2), np.arange(1, D, 2)])

    def wslice(W, rows, dtype=NPBF16):
        # -> [P, EC, ncols] : wT[p, ec, c] = W[rows[c], ec*128 + p]
        wt = W[rows].T.astype(dtype)             # [E, ncols]
        return np.ascontiguousarray(
            wt.reshape(EC, P, len(rows)).transpose(1, 0, 2))

    in_maps = []
    for core in range(N_CORES):
        heads = range(core * HPC, (core + 1) * HPC)
        rows_qk = np.concatenate([h * D + perm for h in heads])
        rows_v = np.concatenate([np.arange(h * D, (h + 1) * D) for h in heads])
        # woT[p, hc, e] = Wo[e, rows_v[hc*128 + p]]
        wo_t = Wo[:, rows_v].T.astype(NPBF16)    # [HD, E]
        wo_t = np.ascontiguousarray(
            wo_t.reshape(HPC, P, E).transpose(1, 0, 2))
        in_maps.append({
            "xTt": xtt,
            "x8t": x8tt,
            "wqT": wslice(Wq * WSCALE, rows_qk, NPF8),
            "wkT": wslice(Wk * WSCALE, rows_qk, NPF8),
            "wvT": wslice(Wv, rows_v),
            "woT": wo_t,
            "rm1": m1,
            "rm2": m2,
            "msk": msk,
        })
    return in_maps


_NC_CACHE = None


def _get_nc():
    global _NC_CACHE
    if _NC_CACHE is None:
        _NC_CACHE = build_nc()
    return _NC_CACHE


def kernel(x, Wq, Wk, Wv, Wo, _want_trace=False):
    in_maps = _prep_in_maps(x, Wq, Wk, Wv, Wo)
    nc = _get_nc()
    trace = _want_trace or bool(os.environ.get("KERNEL_TRACE"))
    res = bass_utils.run_bass_kernel_spmd(
        nc, in_maps, core_ids=list(range(N_CORES)), trace=trace,
    )
    acc = np.zeros((T, E), np.float64)
    for c in range(N_CORES):
        acc += res.results[c]["out"].astype(np.float64)
    outv = acc.astype(np.float32).reshape(B, S, E)
    if _want_trace:
        return outv, res
    return outv



# revision 49
# speedup vs baseline: 1.1595x; 1.1595x over previous
"""Tensor-parallel causal multi-head attention (RoPE) for 8 Trainium2 cores.

Problem: nn_Attention (B=2, S=2048, E=2048, H=16, interleaved-pair RoPE,
causal softmax with 1/sqrt(E) scaling, output projection).

Sharding: tensor-parallel over heads — each of the 8 cores owns 2 heads
(the matching 256 columns of Wq/Wk/Wv and rows of Wo), x is replicated,
and the post-Wo all-reduce is done on the host (sum of 8 partials).

Per-core device pipeline (fp32 psum accumulation everywhere):
  1. Q/K projections run in fp8e4m3 with DoubleRow perf mode (2 k-tiles
     of 128 per pass, 0.5 cyc/row — 2x bf16): x and Wq/Wk are staged in
     fp8 (weights pre-scaled by WSCALE=64 into e4m3's normal range; the
     1/WSCALE^2 rides the softmax exp scale).  The fp8 quantization only
     perturbs attention *scores* (~1% of a prob), not the value path.
     V projection stays bf16 from a bf16 copy of x.  Q^T/K^T are kept in
     transposed layout [D, t]; V in natural layout [t, D].
  2. RoPE on Q^T/K^T psums directly: a_t = psum*m1 and c_t = psum*m2'
     (DVE, m2' pre-swapped on host), half-swap of c_t via two Scalar-queue
     DMAs (a compute-dependent DMA on the Sync queue would head-of-line
     block the x prefetches), final add on GpSimd.
  3. Attention per (batch, head, q-tile of 512) with key chunks processed
     in pairs (one [128,1024] exp per pair on ACT; no max-subtraction:
     |scores/sqrt(E)| <~ 1.5).  Diagonal chunks are causally trimmed and
     masked (DVE).  PV accumulates out^T += V_c^T probs^T on PE.  The
     softmax denominator: diagonal chunks use a bf16 ones-matmul; full
     pairs are cast to fp8 (DVE) and reduced by fp8 DoubleRow ones-matmuls
     batched at q-tile end (0.5 cyc/row over both chunks).
  4. Normalization: deferred psum eviction, reciprocal_approx_fast, scale
     in place.  Output projection from out^T (stationary) -> fp16 partial
     [t, E]; host sums the 8 partials in fp64.

Schedule: projection tiles and attention q-tiles are interleaved (attention
(b, qt) only needs K/V coverage up to tile b*4+qt), with output-projection
chunks streaming one q-tile behind attention; x tiles prefetch 2 ahead on
the pure-load Sync queue; psum evictions alternate Scalar/Vector so neither
queue lags the PE; ~5 us of warm-up matmuls ramp the DVFS clock during the
startup DMA window.
"""

import math
import os
from contextlib import ExitStack

import ml_dtypes
import numpy as np

import concourse.bass as bass
import concourse.mybir as mybir
import concourse.tile as tile
from concourse import bacc, bass_isa, bass_utils

# denominator strategy: "pe" = ones-matmul on TensorE into psum;
# "gpsimd" = accumulate exp chunks + partition_all_reduce on GpSimd
DENOM_MODE = os.environ.get("KERNEL_DENOM", "pe")
# partial-output dtype: bf16 halves the output DMA; host sums in fp64
# partial-output dtype fp16: same 2 bytes/elem as bf16 but 10 mantissa
# bits, so the host-side 8-way partial sum loses less precision
OUT_F16 = os.environ.get("KERNEL_OUT", "f16") == "f16"

# ---------------------------------------------------------------- constants
B, S, E = 2, 2048, 2048
H = 16
N_CORES = 8
HPC = H // N_CORES          # heads per core = 2
D = E // H                  # head dim = 128
T = B * S                   # tokens = 4096
HD = HPC * D                # per-core head dims = 256
ATTN_SCALE = 1.0 / math.sqrt(E)
ROPE_BASE = 10000.0

P = 128
EC = E // P                 # 16 contraction chunks
T_TILE = 512
NT = T // T_TILE            # 8 projection token tiles
QTS = 512                   # attention q-tile size
NQT = S // QTS              # 4 q-tiles per (b, h)
NKC = S // P                # 16 key chunks per batch

BF16 = mybir.dt.bfloat16
F32 = mybir.dt.float32
F8 = mybir.dt.float8e4
F16 = mybir.dt.float16
NPBF16 = ml_dtypes.bfloat16
NPF8 = ml_dtypes.float8_e4m3

# Q/K projections run in fp8e4m3 with DoubleRow (2 k-tiles per pass, 0.5
# cyc/row).  Wq/Wk entries (~N(0, 1/E)) are pre-scaled by WSCALE so they
# land in e4m3's normal range; the 1/WSCALE^2 is folded into the exp scale.
WSCALE = 64.0


# ---------------------------------------------------------------- device IR
def _emit(tc, ctx):
    nc = tc.nc
    xTt = nc.dram_tensor("xTt", [NT, P, EC, T_TILE], BF16, kind="ExternalInput").ap()
    x8t = nc.dram_tensor("x8t", [NT, P, EC, T_TILE], F8, kind="ExternalInput").ap()
    wqT = nc.dram_tensor("wqT", [P, EC, HD], F8, kind="ExternalInput").ap()
    wkT = nc.dram_tensor("wkT", [P, EC, HD], F8, kind="ExternalInput").ap()
    wvT = nc.dram_tensor("wvT", [P, EC, HD], BF16, kind="ExternalInput").ap()
    woT = nc.dram_tensor("woT", [P, HPC, E], BF16, kind="ExternalInput").ap()
    rm1 = nc.dram_tensor("rm1", [P, S], BF16, kind="ExternalInput").ap()
    rm2 = nc.dram_tensor("rm2", [P, S], BF16, kind="ExternalInput").ap()
    msk = nc.dram_tensor("msk", [P, QTS], BF16, kind="ExternalInput").ap()
    out = nc.dram_tensor("out", [T, E], F16 if OUT_F16 else F32,
                         kind="ExternalOutput").ap()

    wpool = ctx.enter_context(tc.tile_pool(name="wpool", bufs=1))
    xpool = ctx.enter_context(tc.tile_pool(name="xpool", bufs=2))
    qkv = ctx.enter_context(tc.tile_pool(name="qkv", bufs=1))
    work = ctx.enter_context(tc.tile_pool(name="work", bufs=3))
    psA = ctx.enter_context(tc.tile_pool(name="psA", bufs=2, space="PSUM"))
    psO = ctx.enter_context(tc.tile_pool(name="psO", bufs=2, space="PSUM"))
    psD = ctx.enter_context(tc.tile_pool(name="psD", bufs=2, space="PSUM"))

    # --- persistent SBUF state
    wq_s = wpool.tile([P, EC, HD], F8)
    wk_s = wpool.tile([P, EC, HD], F8)
    wv_s = wpool.tile([P, EC, HD], BF16)
    wo_s = wpool.tile([P, HPC, E], BF16)
    m1_s = wpool.tile([P, S], BF16)
    m2_s = wpool.tile([P, S], BF16)
    mk_s = wpool.tile([P, QTS], BF16)
    ones_s = wpool.tile([P, P], BF16)
    ones8_s = wpool.tile([P, 2, P], F8)   # DoubleRow ones for fp8 denoms
    # startup-latency ordering: few big descriptors (each ~0.6 us of issue
    # time on the Sync queue), fp8 weights/x first so the first Q chain can
    # run as soon as possible
    xt80 = xpool.tile([P, EC, T_TILE], F8, tag="xt8", bufs=3)
    xt0 = xpool.tile([P, EC, T_TILE], BF16, tag="xt")
    nc.sync.dma_start(wq_s[:], wqT[:])
    nc.sync.dma_start(xt80[:], x8t[0])
    nc.sync.dma_start(wk_s[:], wkT[:])
    nc.any.memset(ones_s[:], 1.0)
    nc.any.memset(ones8_s[:], 1.0)
    # HAM warm-up: ~3.5 µs of dummy matmuls during the startup DMA window
    # so the first real matmuls run at the full 2.4 GHz clock
    warm = psA.tile([P, 512], F32, tag="big", bufs=3,
                    padded_shape=[P, 2 * QTS])
    for i in range(56):
        nc.tensor.matmul(warm[:, 0:P], lhsT=ones_s[:], rhs=ones_s[:],
                         start=(i == 0), stop=(i == 55))
    # rope maps: first token-tile's columns land early so the first rope
    # vector ops don't wait on the full 1 MB map load
    nc.sync.dma_start(m1_s[:, 0:T_TILE], rm1[:, 0:T_TILE])
    nc.sync.dma_start(m2_s[:, 0:T_TILE], rm2[:, 0:T_TILE])
    nc.sync.dma_start(wv_s[:], wvT[:])
    # tile 0's bf16 x in quarters: the V chain's ec=0 matmul only needs the
    # first quarter, so V-proj starts ~4x earlier than with one descriptor
    for q4 in range(4):
        nc.sync.dma_start(xt0[:, 4 * q4:4 * q4 + 4, :],
                          xTt[0, :, 4 * q4:4 * q4 + 4, :])
    # bulk loads that are needed late (wo only in phase C, msk in phase B,
    # map remainders from tile 1 on) are emitted inside the tile loop so
    # the early x-tile prefetches aren't queued behind them

    qT_s = qkv.tile([P, HPC, T], BF16)   # roped Q^T  [d, h, t]
    kT_s = qkv.tile([P, HPC, T], BF16)   # roped K^T
    v_s = qkv.tile([P, T // P, HD], BF16)  # V natural [t%128, t//128, hd]
    oT_s = qkv.tile([P, HPC, T], BF16)   # normalized out^T [d, h, t]

    # ---------------- phase A: projections + RoPE
    # x tiles are prefetched ahead (fp8 tiles two ahead — they gate the Q/K
    # chains; bf16 one ahead), emitted before tile tt's rope/V work floods
    # the Sync queue
    x8tiles = {0: xt80}
    xtiles = {0: xt0}

    def _pf8(tt):
        if tt < NT and tt not in x8tiles:
            nxt8 = xpool.tile([P, EC, T_TILE], F8, tag="xt8", bufs=3)
            nc.sync.dma_start(nxt8[:], x8t[tt])
            x8tiles[tt] = nxt8

    def _pf16(tt):
        if tt < NT and tt not in xtiles:
            nxt = xpool.tile([P, EC, T_TILE], BF16, tag="xt")
            nc.sync.dma_start(nxt[:], xTt[tt])
            xtiles[tt] = nxt

    def _tileA(tt):
        ts0 = tt * T_TILE
        xt8 = x8tiles.pop(tt)
        xt = xtiles.pop(tt)
        _pf8(tt + 1)
        _pf16(tt + 1)
        _pf8(tt + 2)
        if tt == 0:
            nc.sync.dma_start(m1_s[:, T_TILE:], rm1[:, T_TILE:])
            nc.sync.dma_start(m2_s[:, T_TILE:], rm2[:, T_TILE:])
            nc.sync.dma_start(mk_s[:], msk[:])
        elif tt == 2:
            nc.sync.dma_start(wo_s[:], woT[:])

        for w_s, dst in ((wq_s, qT_s), (wk_s, kT_s)):
            psb = psA.tile([P, 2 * T_TILE], F32, tag="big", bufs=3)
            for hs in range(HPC):
                ps = psb[:, hs * T_TILE:(hs + 1) * T_TILE]
                for e2 in range(EC // 2):
                    nc.tensor.matmul(
                        ps,
                        lhsT=w_s[:, 2 * e2:2 * e2 + 2, hs * P:(hs + 1) * P],
                        rhs=xt8[:, 2 * e2:2 * e2 + 2, :],
                        start=(e2 == 0),
                        stop=(e2 == EC // 2 - 1),
                        perf_mode=mybir.MatmulPerfMode.DoubleRow,
                    )
                # RoPE: e = [x1; x2];  out = e*[cos;cos] + swap64(e)*[-sin;sin]
                #     = e*m1 + swap64(e*m2')   with m2' = swap64(m2)
                # Both multiplies read the psum directly (no eviction copy);
                # the half-swap DMAs ride the Scalar queue right behind their
                # producer (a compute-dependent DMA on the Sync queue would
                # head-of-line block the x prefetches), and the final add
                # runs on the otherwise-idle GpSimd — qT/kT have a full
                # tile of slack before attention reads them.
                ms0 = ts0 % S  # the rope maps are identical across batches
                c_t = work.tile([P, T_TILE], BF16, tag="rope_c")
                nc.vector.tensor_mul(c_t[:], ps, m2_s[:, ms0:ms0 + T_TILE])
                a_t = work.tile([P, T_TILE], BF16, tag="rope_a")
                nc.vector.tensor_mul(a_t[:], ps, m1_s[:, ms0:ms0 + T_TILE])
                b_t = work.tile([P, T_TILE], BF16, tag="rope_b")
                nc.scalar.dma_start(b_t[0:64, :], c_t[64:128, :])
                nc.scalar.dma_start(b_t[64:128, :], c_t[0:64, :])
                nc.gpsimd.tensor_add(
                    out=dst[:, hs, ts0:ts0 + T_TILE], in0=a_t[:], in1=b_t[:])

        for sp in range(T_TILE // P // 2):
            psb = psA.tile([P, 2 * HD], F32, tag="big", bufs=3,
                           padded_shape=[P, 2 * QTS])
            for k in range(2):
                sub = 2 * sp + k
                for ec in range(EC):
                    nc.tensor.matmul(
                        psb[:, k * HD:(k + 1) * HD],
                        lhsT=xt[:, ec, sub * P:(sub + 1) * P],
                        rhs=wv_s[:, ec, :],
                        start=(ec == 0),
                        stop=(ec == EC - 1),
                    )
            nc.scalar.copy(
                v_s[:, tt * (T_TILE // P) + 2 * sp:
                    tt * (T_TILE // P) + 2 * sp + 2, :], psb[:])

    # ---------------- phase C helper: output projection for one 128-token
    # chunk; interleaved into phase B with a one-q-tile lag so the out DMA
    # streams during attention instead of piling up at the end
    def _phaseC_chunk(b, tch, last):
        t0 = b * S + tch * P
        stage = work.tile([P, E], F16 if OUT_F16 else F32, tag="wo_out")
        for ep in range(E // 1024):
            wps = psA.tile([P, 1024], F32, tag="big", bufs=3)
            for k in range(2):
                es = 2 * ep + k
                for hc in range(HPC):
                    nc.tensor.matmul(
                        wps[:, k * 512:(k + 1) * 512],
                        lhsT=oT_s[:, hc, t0:t0 + P],
                        rhs=wo_s[:, hc, es * 512:(es + 1) * 512],
                        start=(hc == 0),
                        stop=(hc == HPC - 1),
                    )
            # alternate eviction engines so neither queue lags the PE
            if ep == 0:
                nc.scalar.copy(stage[:, ep * 1024:(ep + 1) * 1024], wps[:])
            else:
                nc.vector.tensor_copy(
                    out=stage[:, ep * 1024:(ep + 1) * 1024], in_=wps[:])
            if last:
                # drain the final tile per-slice to shorten the tail
                nc.sync.dma_start(
                    out[t0:t0 + P, ep * 1024:(ep + 1) * 1024],
                    stage[:, ep * 1024:(ep + 1) * 1024])
        if not last:
            nc.sync.dma_start(out[t0:t0 + P, :], stage[:])

    pendingC = []  # (b, tch) chunks whose oT is complete but Wo not emitted

    def _flushC(upto):
        while len(pendingC) > upto:
            cb, ctch = pendingC.pop(0)
            _phaseC_chunk(cb, ctch, last=(cb == B - 1 and ctch == S // P - 1))

    # ---------------- phase B: attention for one (batch, head, q-tile)
    def _attn(b, hs, qt):
        qTb = qT_s[:, hs, b * S:(b + 1) * S]
        kTb = kT_s[:, hs, b * S:(b + 1) * S]
        if True:
            if True:
                q0 = qt * QTS
                nck = (q0 + QTS) // P  # causal: key chunks 0..nck-1
                ops = psO.tile([P, QTS], F32, tag="outT", bufs=1)
                if DENOM_MODE == "pe":
                    dps = psD.tile([P, QTS], F32, tag="den", bufs=1)
                else:
                    acc = work.tile([P, QTS], F32, tag="acc", bufs=2)
                # full (untrimmed) pairs contribute to the denominator via a
                # single fp8 DoubleRow ones-matmul per pair (0.5 cyc/row over
                # both chunks), batched after the pair loop; only the 4
                # diagonal chunks keep the bf16 per-chunk ones-matmul.
                # fp8 quantization of exp values is harmless here: the
                # denominator errors are incoherent over >=512 keys.
                dr8s = []
                first_inline = max(0, nck - 4)
                for pp in range(nck // 2):
                    cc = (2 * pp, 2 * pp + 1)
                    # causal trim: diagonal chunk j (=c-(nck-4)) only has
                    # valid queries q >= q0 + 128*j  ->  width 512-128*j
                    jj = [max(0, c - (nck - 4)) for c in cc]
                    off = [128 * j for j in jj]
                    sps = psA.tile([P, 2 * QTS], F32, tag="big", bufs=3)
                    for half, c in enumerate(cc):
                        nc.tensor.matmul(
                            sps[:, half * QTS + off[half]:(half + 1) * QTS],
                            lhsT=kTb[:, c * P:(c + 1) * P],
                            rhs=qTb[:, q0 + off[half]:q0 + QTS],
                            start=True,
                            stop=True,
                        )
                    ex = work.tile([P, 2 * QTS], BF16, tag="exps", bufs=6)
                    exp_scale = ATTN_SCALE / (WSCALE * WSCALE)
                    if off[0] == 0 and off[1] == 0:
                        nc.scalar.activation(
                            ex[:], sps[:], mybir.ActivationFunctionType.Exp,
                            scale=exp_scale,
                        )
                    else:
                        for half in range(2):
                            sl = slice(half * QTS + off[half], (half + 1) * QTS)
                            nc.scalar.activation(
                                ex[:, sl], sps[:, sl],
                                mybir.ActivationFunctionType.Exp,
                                scale=exp_scale,
                            )
                    full_pair = DENOM_MODE == "pe" and cc[1] < nck - 4
                    if full_pair:
                        ex8 = work.tile([P, 2, QTS], F8, tag="exps8", bufs=6)
                        nc.vector.tensor_copy(out=ex8[:, 0, :], in_=ex[:, 0:QTS])
                        nc.vector.tensor_copy(out=ex8[:, 1, :], in_=ex[:, QTS:])
                        dr8s.append(ex8)
                    for half, c in enumerate(cc):
                        w = QTS - off[half]
                        exh = ex[:, half * QTS + off[half]:(half + 1) * QTS]
                        if c >= nck - 4:
                            # intra-block triangle: reuse the j=0 mask, width w
                            nc.vector.tensor_mul(exh, exh, mk_s[:, :w])
                        nc.tensor.matmul(
                            ops[:, off[half]:QTS],
                            lhsT=v_s[:, b * NKC + c, hs * P:(hs + 1) * P],
                            rhs=exh,
                            start=(c == 0),
                            stop=(c == nck - 1),
                        )
                        if DENOM_MODE == "pe":
                            if not full_pair:
                                nc.tensor.matmul(
                                    dps[:, off[half]:QTS],
                                    lhsT=ones_s[:],
                                    rhs=exh,
                                    start=(c == first_inline),
                                    stop=(c == nck - 1 and nck == 4),
                                )
                        elif c == 0:
                            nc.gpsimd.tensor_copy(out=acc[:], in_=exh)
                        else:
                            accs = acc[:, off[half]:]
                            nc.gpsimd.tensor_add(out=accs, in0=accs, in1=exh)
                # batched fp8 DoubleRow denominator matmuls for the full
                # pairs (the inline diag chunks ran first and set start)
                for di, ex8 in enumerate(dr8s):
                    nc.tensor.matmul(
                        dps[:],
                        lhsT=ones8_s[:],
                        rhs=ex8[:],
                        start=False,
                        stop=(di == len(dr8s) - 1),
                        perf_mode=mybir.MatmulPerfMode.DoubleRow,
                        skip_group_check=True,
                    )
                # normalize: oT = ops * (1/denom), denom replicated to all
                # 128 partitions (by the ones-matmul / partition_all_reduce)
                oslice = oT_s[:, hs, b * S + q0: b * S + q0 + QTS]
                rb = work.tile([P, QTS], F32, tag="recipb", bufs=2)
                if DENOM_MODE == "pe":
                    # deferred eviction: the copy frees the psO bank without
                    # waiting on the reciprocal, then scale in place
                    nc.vector.tensor_copy(out=oslice, in_=ops[:])
                    nc.vector.reciprocal_approx_fast(out=rb[:], in_=dps[:])
                    nc.vector.tensor_mul(oslice, oslice, rb[:])
                else:
                    # deferred normalization: release the psum bank with an
                    # unnormalized eviction; scale in place once the (slow,
                    # off-critical-path) GpSimd denominator lands
                    nc.vector.tensor_copy(out=oslice, in_=ops[:])
                    red = work.tile([P, QTS], F32, tag="red")
                    nc.gpsimd.partition_all_reduce(
                        red[:], acc[:], P, bass_isa.ReduceOp.add)
                    nc.vector.reciprocal_approx_fast(out=rb[:], in_=red[:])
                    nc.vector.tensor_mul(oslice, oslice, rb[:])

    # ---------------- schedule: A-tiles interleaved with attention q-tiles
    # attention (b, qt) needs K/V coverage of tiles b*NQT..b*NQT+qt only, so
    # it can start as soon as those projection tiles are done; the early
    # A-tiles' DMA debt is hidden behind attention compute that needs no
    # new input, and phase-C chunks (one q-tile of lag) stream the output
    _tileA(0)
    _tileA(1)
    _tileA(2)
    for b in range(B):
        for qt in range(NQT):
            for hs in range(HPC):
                _attn(b, hs, qt)
            # both heads' oT for tokens [q0, q0+QTS) are now final; queue
            # their output-projection chunks, flush with one q-tile of lag
            # so the normalize has time to land
            pendingC.extend(
                (b, tc) for tc in range(qt * (QTS // P),
                                        (qt + 1) * (QTS // P)))
            _flushC(QTS // P)
            t_next = b * NQT + qt + 3
            if t_next < NT:
                _tileA(t_next)
    _flushC(0)


def build_nc():
    nc = bacc.Bacc("TRN2", target_bir_lowering=False, debug=False, num_devices=1)
    with tile.TileContext(nc) as tc, ExitStack() as ctx:
        _emit(tc, ctx)
    nc.compile()
    return nc


# ---------------------------------------------------------------- host prep
def _rope_maps():
    half = D // 2
    inv = 1.0 / (ROPE_BASE ** (np.arange(half, dtype=np.float64) / half))
    ang = np.arange(S, dtype=np.float64)[None, :] * inv[:, None]  # [64, S]
    cos = np.cos(ang)
    sin = np.sin(ang)
    m1 = np.concatenate([cos, cos], axis=0)   # [128, S] multiplies e=[x1;x2]
    # m2' = swap64(m2): multiplies e BEFORE the half-swap, so that
    # swap64(e*m2') == swap64(e)*m2 with m2 = [-sin; sin]
    m2 = np.concatenate([sin, -sin], axis=0)
    return (np.ascontiguousarray(m1.astype(NPBF16)),
            np.ascontiguousarray(m2.astype(NPBF16)))


def _masks():
    kk = np.arange(P)[:, None]
    qq = np.arange(QTS)[None, :]
    return np.ascontiguousarray((kk <= qq).astype(NPBF16))  # [128, 512]


def _prep_in_maps(x, Wq, Wk, Wv, Wo):
    x = np.asarray(x, np.float32)
    Wq = np.asarray(Wq, np.float32)
    Wk = np.asarray(Wk, np.float32)
    Wv = np.asarray(Wv, np.float32)
    Wo = np.asarray(Wo, np.float32)

    # x^T tiled: [NT, 128, EC, T_TILE];  xT[e, t] = x[t, e]
    xTf = x.reshape(T, E).T                                    # [E, T] f32
    xT = xTf.astype(NPBF16)
    xtt = xT.reshape(EC, P, NT, T_TILE).transpose(2, 1, 0, 3)  # [NT,P,EC,TT]
    xtt = np.ascontiguousarray(xtt)
    x8 = xTf.astype(NPF8)
    x8tt = np.ascontiguousarray(
        x8.reshape(EC, P, NT, T_TILE).transpose(2, 1, 0, 3))

    m1, m2 = _rope_maps()
    msk = _masks()

    # de-interleave perm for RoPE pair-contiguity
    perm = np.concatenate([np.arange(0, D, 2), np.arange(1, D, 2)])

    def wslice(W, rows, dtype=NPBF16):
        # -> [P, EC, ncols] : wT[p, ec, c] = W[rows[c], ec*128 + p]
        wt = W[rows].T.astype(dtype)             # [E, ncols]
        return np.ascontiguousarray(
            wt.reshape(EC, P, len(rows)).transpose(1, 0, 2))

    in_maps = []
    for core in range(N_CORES):
        heads = range(core * HPC, (core + 1) * HPC)
        rows_qk = np.concatenate([h * D + perm for h in heads])
        rows_v = np.concatenate([np.arange(h * D, (h + 1) * D) for h in heads])
        # woT[p, hc, e] = Wo[e, rows_v[hc*128 + p]]
        wo_t = Wo[:, rows_v].T.astype(NPBF16)    # [HD, E]
        wo_t = np.ascontiguousarray(
            wo_t.reshape(HPC, P, E).transpose(1, 0, 2))
        in_maps.append({
            "xTt": xtt,
            "x8t": x8tt,
            "wqT": wslice(Wq * WSCALE, rows_qk, NPF8),
            "wkT": wslice(Wk * WSCALE, rows_qk, NPF8),
            "wvT": wslice(Wv, rows_v),
            "woT": wo_t,
            "rm1": m1,
            "rm2": m2,
            "msk": msk,
        })
    return in_maps


_NC_CACHE = None


def _get_nc():
    global _NC_CACHE
    if _NC_CACHE is None:
        _NC_CACHE = build_nc()
    return _NC_CACHE


def kernel(x, Wq, Wk, Wv, Wo, _want_trace=False):
    in_maps = _prep_in_maps(x, Wq, Wk, Wv, Wo)
    nc = _get_nc()
    trace = _want_trace or bool(os.environ.get("KERNEL_TRACE"))
    res = bass_utils.run_bass_kernel_spmd(
        nc, in_maps, core_ids=list(range(N_CORES)), trace=trace,
    )
    acc = np.zeros((T, E), np.float64)
    for c in range(N_CORES):
        acc += res.results[c]["out"].astype(np.float64)
    outv = acc.astype(np.float32).reshape(B, S, E)
    if _want_trace:
        return outv, res
    return outv

